# revision 25
# baseline (speedup 1.0000x reference)
"""CategoryAttention (softmax over heads axis) on 8 Trainium2 cores.

Sharding: B*L = 2*2048 = 4096 query rows split 8 ways (512 rows/core),
core c handles batch b=c//4, query rows [(c%4)*512, (c%4+1)*512).
The head-axis softmax is local to each (q,k) position, so attention
needs no communication — and K/V projections are not recomputed per
core either: each core projects only HALF of its batch's K/V rows
(even c%2 takes k rows [0,1024), odd takes [1024,2048)) and the halves
are exchanged with 2-rank AllGathers within pairs (0,1),(2,3),(4,5),
(6,7).  Each half is gathered in two 1MB quarters issued as soon as
they are projected, so the collectives overlap the remaining
projections and the first 8 attention k-tiles; gathered quarters land
at rank-invariant global k positions, so the kernel never needs to
know its own rank.  Attention processes k-tiles in arrival order
(0-3, 8-11 from quarter 0; 4-7, 12-15 from quarter 1).

Engine budget per k-tile (the inner loop is overhead-dominated, so few
large ops): PE 6.8us (energy+AV matmuls, AV interleaved between energy
groups so ACT-gated PSUM waits are filled); ACT 8.3us (exp straight
from PSUM in a 4-2-4-2-4 head pattern — alternating 4-bank and 2-bank
psum tiles double-buffer inside the 8-bank budget); DVE ~8us (head-sum
reduce, reciprocal, most of the normalize multiply, AV psum drains);
GPSIMD (2us/op dispatch overhead — give it few big ops) the f32->bf16
recip cast and the rest of the normalize.
"""

import numpy as np
from contextlib import ExitStack

import concourse.bass as bass
import concourse.tile as tile
from concourse import bacc, mybir
from concourse.bass_utils import run_bass_kernel_spmd

F32 = mybir.dt.float32
F32R = mybir.dt.float32r
BF16 = mybir.dt.bfloat16

N_CORES = 8
P = 128
D = 1024          # d_model
S = D // P        # 8 subtiles of the contraction dim
H = 16            # heads
HD = 64           # head dim
B = 2
L = 2048
LQ = L * B // N_CORES   # 512 query rows per core
LK = L                  # key rows per batch
LKH = LK // 2           # 1024 key rows projected per core (own half)
KTS = 128               # k tile
NKT = LK // KTS         # 16
KC = 2                  # k-tiles per AV psum accumulation chunk
SCALE = 1.0 / np.sqrt(HD)

EXP_PAT = (4, 2, 4, 2, 4)   # heads per ACT exp-from-psum call
ND = 16                     # heads normalized on DVE (rest GPSIMD)
# process k-tiles in gather-arrival order: quarter 0 covers global
# k-tiles {0..3, 8..11}, quarter 1 covers {4..7, 12..15}
KT_ORDER = list(range(16))

GROUPS = [[0, 1], [2, 3], [4, 5], [6, 7]]


def _build(has_bias):
    nc = bacc.Bacc("TRN2", target_bir_lowering=False, debug=False,
                   num_devices=N_CORES)

    def din(name, shape, dt):
        return nc.dram_tensor(name, shape, dt, kind="ExternalInput").ap()

    qT_d = din("qT", (P, S * LQ), BF16)
    kT_d = din("kTh", (P, S * LKH), BF16)
    vT_d = din("vTh", (P, S * LKH), BF16)
    wq_d = din("wq", (P, S * D), BF16)
    wk_d = din("wk", (P, S * D), BF16)
    wv_d = din("wv", (P, S * D), BF16)
    wo_d = din("wo", (P, S * D), F32R)
    goff_d = din("goff", (P, 1), mybir.dt.int32)
    bias_d = {}
    for nm in ("bq", "bk", "bv", "bo"):
        if has_bias[nm]:
            bias_d[nm] = din(nm, (1, D), F32)
    outT_d = nc.dram_tensor("outT", (P, S * LQ), F32, kind="ExternalOutput").ap()

    qT_ap = qT_d.rearrange("p (s q) -> p s q", s=S)
    kT_ap = kT_d.rearrange("p (s k) -> p s k", s=S)
    vT_ap = vT_d.rearrange("p (s k) -> p s k", s=S)
    wq_ap = wq_d.rearrange("p (h s o) -> p h s o", h=2, s=S)
    wk_ap = wk_d.rearrange("p (h s o) -> p h s o", h=2, s=S)
    wv_ap = wv_d.rearrange("p (h s o) -> p h s o", h=2, s=S)
    wo_ap = wo_d.rearrange("p (h s o) -> p h s o", h=2, s=S)
    outT_ap = outT_d.rearrange("p (j q) -> p j q", j=S)

    with tile.TileContext(nc) as tc, ExitStack() as ctx:
        const_pool = ctx.enter_context(tc.tile_pool(name="const", bufs=1))
        qt_pool = ctx.enter_context(tc.tile_pool(name="QT", bufs=1))
        kt_pool = ctx.enter_context(tc.tile_pool(name="KT", bufs=1))
        v_pool = ctx.enter_context(tc.tile_pool(name="V", bufs=1))
        dram_pool = ctx.enter_context(tc.tile_pool(name="dram", bufs=1,
                                                   space="DRAM"))

        any_bias = any(has_bias.values())
        ones_t = None
        if any_bias:
            ones_t = const_pool.tile([1, LKH], F32, tag="ones")
            nc.vector.memset(ones_t[:], 1.0)
        bias_t = {}
        for nm, d_ap in bias_d.items():
            t = const_pool.tile([1, D], F32, tag=f"bias_{nm}")
            nc.sync.dma_start(t[:], d_ap)
            bias_t[nm] = t

        QT_sb = qt_pool.tile([P, S, LQ], BF16)
        KT_sb = kt_pool.tile([P, S, LK], BF16)
        V_sb = v_pool.tile([P, NKT, D], BF16)

        # pair-shared HBM: cores (2k, 2k+1) alias these tensors, so a
        # plain DMA write lands in the peer's address space too.  Each core
        # scatters its K/V half to its own slot (row indices fed from the
        # host via goff — rank-dependent data, rank-oblivious program),
        # then a 2-byte pair AllGather acts as the write barrier before
        # the canonical-layout read-back.
        ksh = dram_pool.tile([2, P, S * LKH], BF16, tag="ksh", name="ksh",
                             addr_space="Shared")
        vsh = dram_pool.tile([2, P, S * LKH], BF16, tag="vsh", name="vsh",
                             addr_space="Shared")
        kbar_i = dram_pool.tile([1, 2], mybir.dt.uint8, tag="kbi", name="kbi")
        kbar_o = dram_pool.tile([2, 2], mybir.dt.uint8, tag="kbo", name="kbo")
        vbar_i = dram_pool.tile([1, 2], mybir.dt.uint8, tag="vbi", name="vbi")
        vbar_o = dram_pool.tile([2, 2], mybir.dt.uint8, tag="vbo", name="vbo")
        goff_t = const_pool.tile([P, 1], mybir.dt.int32, tag="goff")
        nc.sync.dma_start(goff_t[:], goff_d)
        bar_sb = const_pool.tile([1, 2], mybir.dt.uint8, tag="barsb")
        bscr = const_pool.tile([1, 4], mybir.dt.uint8, tag="bscr")
        nc.vector.memset(bar_sb[:], 0)

        def bias_mm(ps_t, bias_name, o0, n_sz, o_on_partitions):
            if o_on_partitions:
                nc.tensor.matmul(ps_t, lhsT=bias_t[bias_name][0:1, o0:o0 + P],
                                 rhs=ones_t[0:1, :n_sz], start=False, stop=True)
            else:
                nc.tensor.matmul(ps_t, lhsT=ones_t[0:1, 0:P],
                                 rhs=bias_t[bias_name][0:1, o0:o0 + n_sz],
                                 start=False, stop=True)

        # ------- phase 1: K/V quarter projections + gathers, Q proj -------
        with tc.tile_pool(name="stream", bufs=4) as spool, \
             tc.tile_pool(name="stage", bufs=1) as stpool, \
             tc.tile_pool(name="ppsum", bufs=3, space="PSUM") as ppsum:

            def pp_tile():
                return ppsum.tile([P, 2, 512], F32, tag="pp", name="pp")

            def stream_w(src_ap):
                t = spool.tile([P, S, 512], BF16, tag="w")
                nc.sync.dma_start(t[:], src_ap)
                return t

            def stream_in(src_ap):
                t = spool.tile([P, S, 512], BF16, tag="in")
                nc.sync.dma_start(t[:], src_ap)
                return t

            # K^T half: out o on partitions, own k cols moving
            wk_h = [stream_w(wk_ap[:, 0])]
            kin0 = stream_in(kT_ap[:, :, 0:512])
            wk_h.append(stream_w(wk_ap[:, 1]))
            wv_h = None
            Kst = stpool.tile([P, S, LKH], BF16, tag="kst")
            Vst = stpool.tile([P, S, LKH], BF16, tag="vst")
            for qh in range(2):
                kin = kin0 if qh == 0 else \
                    stream_in(kT_ap[:, :, qh * 512:(qh + 1) * 512])
                for jp in range(4):
                    ps = pp_tile()
                    for jj in range(2):
                        j = jp * 2 + jj
                        w_t = wk_h[j // 4]
                        jl = j % 4
                        for s in range(S):
                            nc.tensor.matmul(
                                ps[:, jj, :],
                                lhsT=w_t[:, s, jl * P:(jl + 1) * P],
                                rhs=kin[:, s, :],
                                start=(s == 0),
                                stop=(s == S - 1 and not has_bias["bk"]),
                            )
                        if has_bias["bk"]:
                            bias_mm(ps[:, jj, :], "bk", j * P, 512, True)
                    nc.scalar.copy(
                        Kst[:, jp * 2:(jp + 1) * 2, qh * 512:(qh + 1) * 512],
                        ps[:])
            # scatter own K half into the pair-shared buffer, then barrier
            nc.gpsimd.indirect_dma_start(
                out=ksh[:].rearrange("a p n -> (a p) n"),
                out_offset=bass.IndirectOffsetOnAxis(ap=goff_t[:, 0:1], axis=0),
                in_=Kst[:].rearrange("p s k -> p (s k)"),
                in_offset=None,
            )

            # V half: own k rows on partitions, d moving
            for qh in range(2):
                if wv_h is None:
                    wv_h = [stream_w(wv_ap[:, wh]) for wh in range(2)]
                vin = stream_in(vT_ap[:, :, qh * 512:(qh + 1) * 512])
                for kc in range(4):
                    ktc = qh * 4 + kc
                    ps = pp_tile()
                    for t in range(2):
                        for s in range(S):
                            nc.tensor.matmul(
                                ps[:, t, :],
                                lhsT=vin[:, s, kc * P:(kc + 1) * P],
                                rhs=wv_h[t][:, s, :],
                                start=(s == 0),
                                stop=(s == S - 1 and not has_bias["bv"]),
                            )
                        if has_bias["bv"]:
                            bias_mm(ps[:, t, :], "bv", t * 512, 512, False)
                    nc.vector.tensor_copy(Vst[:, ktc, :],
                                          ps[:].rearrange("p a b -> p (a b)"))
            nc.gpsimd.indirect_dma_start(
                out=vsh[:].rearrange("a p n -> (a p) n"),
                out_offset=bass.IndirectOffsetOnAxis(ap=goff_t[:, 0:1], axis=0),
                in_=Vst[:].rearrange("p s k -> p (s k)"),
                in_offset=None,
            )
            nc.gpsimd.dma_start(vbar_i[:], bar_sb[:])
            nc.gpsimd.collective_compute(
                "AllGather", mybir.AluOpType.bypass, replica_groups=GROUPS,
                ins=[vbar_i.opt()], outs=[vbar_o.opt()])
            # reading the collective's output puts a wait on the GP queue,
            # fencing the readbacks below behind the pair barrier
            nc.gpsimd.dma_start(bscr[:],
                                vbar_o[:].rearrange("a b -> (a b)")[None, :])
            for r in range(2):
                nc.gpsimd.dma_start(
                    KT_sb[:, :, r * LKH:(r + 1) * LKH],
                    ksh[r].rearrange("p (s k) -> p s k", s=S))
            for r in range(2):
                nc.gpsimd.dma_start(
                    V_sb[:, r * 8:(r + 1) * 8, :],
                    vsh[r].rearrange("p (c d) -> p c d", c=S))

            # Q^T
            wq_h = [stream_w(wq_ap[:, wh]) for wh in range(2)]
            qin = stream_in(qT_ap)
            for jp in range(4):
                ps = pp_tile()
                for jj in range(2):
                    j = jp * 2 + jj
                    w_t = wq_h[j // 4]
                    jl = j % 4
                    for s in range(S):
                        nc.tensor.matmul(
                            ps[:, jj, :],
                            lhsT=w_t[:, s, jl * P:(jl + 1) * P],
                            rhs=qin[:, s, :],
                            start=(s == 0),
                            stop=(s == S - 1 and not has_bias["bq"]),
                        )
                    if has_bias["bq"]:
                        bias_mm(ps[:, jj, :], "bq", j * P, LQ, True)
                nc.scalar.copy(QT_sb[:, jp * 2:(jp + 1) * 2, :], ps[:])

        # ---------------- phase 2: attention (q = 512) ----------------
        ctx_pool = ctx.enter_context(tc.tile_pool(name="ctx", bufs=1))
        e_psum = ctx.enter_context(tc.tile_pool(name="epsum", bufs=1, space="PSUM"))
        av_psum = ctx.enter_context(tc.tile_pool(name="avpsum", bufs=1, space="PSUM"))
        ctx_sb = ctx_pool.tile([P, S, LQ], F32R)
        # attention SBUF pools are phase-scoped so the O-projection pools
        # below can reuse their space; attn gets a 5th buffer for pipeline
        # elasticity (4 is the strict minimum and causes periodic drains)
        ph2 = ExitStack()
        attn_pool = ph2.enter_context(tc.tile_pool(name="attn", bufs=5))
        tree_pool = ph2.enter_context(tc.tile_pool(name="tree", bufs=2))
        den_pool = ph2.enter_context(tc.tile_pool(name="den", bufs=1))
        r_pool = ph2.enter_context(tc.tile_pool(name="r", bufs=2))
        rb_pool = ph2.enter_context(tc.tile_pool(name="rb", bufs=1))
        rbx_pool = ph2.enter_context(tc.tile_pool(name="rbx", bufs=1))

        pend_norm = []  # [(attn_t, r32)] normalize deferred one k-tile

        def flush_norm():
            """Emit rb/rbx/norm for the previous k-tile.  Deferring these a
            tile keeps the GPSIMD queue from blocking on a just-issued DVE
            reciprocal (the cross-engine ping-pong that paced the loop)."""
            if not pend_norm:
                return
            attn_p, r32_p = pend_norm.pop()
            rb = rb_pool.tile([P, LQ], BF16, tag="rb")
            nc.gpsimd.tensor_copy(rb[:], r32_p[:])
            # replicate 1/den 8x via DMA (stride-0 broadcast APs knock the
            # DVE into a slow generic mode; a physical copy keeps the
            # normalize multiplies on the fast fully-contiguous path)
            rbx = rbx_pool.tile([P, 8, LQ], BF16, tag="rbx")
            nc.gpsimd.dma_start(rbx[:], rb[:, None, :].to_broadcast((P, 8, LQ)))
            for hb in range(2):
                nc.vector.tensor_mul(
                    attn_p[:, hb * 8:(hb + 1) * 8, :],
                    attn_p[:, hb * 8:(hb + 1) * 8, :],
                    rbx[:])

        def softmax_kt(kt, av_jobs):
            """Energy -> exp (ACT, straight from psum) -> den -> attn.

            av_jobs: deferred AV emitters for the previous chunk, spliced
            between energy groups so the in-order PE queue has matmuls to
            run while ACT drains each psum tile.
            """
            flush_norm()
            attn_t = attn_pool.tile([P, H, LQ], BF16, tag="attn")
            tr = tree_pool.tile([P, 8, LQ], BF16, tag="t8")
            h0 = 0
            for gi, gsz in enumerate(EXP_PAT):
                eps = e_psum.tile([P, gsz, LQ], F32, tag=f"e{gsz}{gi % 2}")
                for hh in range(gsz):
                    h = h0 + hh
                    j2, p0 = h // 2, HD * (h % 2)
                    nc.tensor.matmul(
                        eps[:, hh, :],
                        lhsT=KT_sb[p0:p0 + HD, j2, kt * KTS:(kt + 1) * KTS],
                        rhs=QT_sb[p0:p0 + HD, j2, :],
                        start=True,
                        stop=True,
                    )
                nc.scalar.activation(attn_t[:, h0:h0 + gsz, :], eps[:],
                                     mybir.ActivationFunctionType.Exp,
                                     scale=float(SCALE))
                h0 += gsz
                if gi == 2:
                    # heads 0-7 exp'd: start their half-tree early (DVE —
                    # GPSIMD degrades badly under SBUF port contention)
                    with nc.allow_low_precision(reason="bf16 head-sum tree"):
                        nc.vector.tensor_add(tr[:, 0:4, :], attn_t[:, 0:4, :],
                                             attn_t[:, 4:8, :])
                if gi in (1, 3) and av_jobs:
                    av_jobs.pop(0)()
            while av_jobs:
                av_jobs.pop(0)()
            with nc.allow_low_precision(reason="bf16 head-sum tree"):
                nc.vector.tensor_add(tr[:, 4:8, :], attn_t[:, 8:12, :],
                                     attn_t[:, 12:16, :])
                nc.vector.tensor_add(tr[:, 0:4, :], tr[:, 0:4, :],
                                     tr[:, 4:8, :])
                nc.vector.tensor_add(tr[:, 4:6, :], tr[:, 0:2, :],
                                     tr[:, 2:4, :])
            den = den_pool.tile([P, LQ], F32, tag="den")
            nc.vector.tensor_add(den[:], tr[:, 4, :], tr[:, 5, :])
            r32 = r_pool.tile([P, LQ], F32, tag="r")
            nc.vector.reciprocal_approx_fast(r32[:], den[:])
            pend_norm.append((attn_t, r32))
            return attn_t

        def av_group(u, kts, attn_list, first):
            """One avp tile: heads 4u..4u+3, full q, over the KC k-tiles."""
            avp = av_psum.tile([P, 2, LQ], F32, tag="av")
            for ci, kt in enumerate(kts):
                for hh in range(4):
                    h = 4 * u + hh
                    i, p0 = hh // 2, HD * (hh % 2)
                    nc.tensor.matmul(
                        avp[p0:p0 + HD, i, :],
                        lhsT=V_sb[:, kt, h * HD:(h + 1) * HD],
                        rhs=attn_list[ci][:, h, :],
                        start=(ci == 0),
                        stop=(ci == len(kts) - 1),
                    )
            if first:
                nc.vector.tensor_copy(ctx_sb[:, 2 * u:2 * u + 2, :],
                                      avp[:, :, :])
            else:
                nc.vector.tensor_add(ctx_sb[:, 2 * u:2 * u + 2, :],
                                     ctx_sb[:, 2 * u:2 * u + 2, :],
                                     avp[:, :, :])

        prev = None  # (kts, attn_list)
        for ch in range(NKT // KC):
            kts = KT_ORDER[ch * KC:(ch + 1) * KC]
            cur = []
            for ci in range(KC):
                jobs = []
                if prev is not None:
                    pk, pl = prev
                    first = (ch == 1)
                    jobs = [
                        (lambda u=u, pk=pk, pl=pl, first=first:
                         av_group(u, pk, pl, first))
                        for u in (2 * ci, 2 * ci + 1)
                    ]
                cur.append(softmax_kt(kts[ci], jobs))
            prev = (kts, cur)
        flush_norm()
        for u in range(4):
            av_group(u, prev[0], prev[1], False)

        ph2.close()

        # ---------------- phase 3: output projection ----------------
        osb_pool = ctx.enter_context(tc.tile_pool(name="osb", bufs=2))
        wo_pool = ctx.enter_context(tc.tile_pool(name="wo", bufs=1))
        for j4 in range(2):
            woh = wo_pool.tile([P, S, 512], F32R, tag="wo")
            nc.sync.dma_start(woh[:], wo_ap[:, j4])
            po = e_psum.tile([P, 4, LQ], F32, tag="e40")
            for jj in range(4):
                j = j4 * 4 + jj
                for s in range(S):
                    nc.tensor.matmul(
                        po[:, jj, :],
                        lhsT=woh[:, s, jj * P:(jj + 1) * P],
                        rhs=ctx_sb[:, s, :],
                        start=(s == 0),
                        stop=(s == S - 1 and not has_bias["bo"]),
                    )
                if has_bias["bo"]:
                    bias_mm(po[:, jj, :], "bo", j * P, LQ, True)
            osb = osb_pool.tile([P, 4, LQ], F32, tag="osb")
            nc.scalar.copy(osb[:], po[:])
            nc.sync.dma_start(outT_ap[:, j4 * 4:(j4 + 1) * 4, :], osb[:])

    nc.compile()
    return nc


_cache = {}


def _get_program(has_bias):
    key = tuple(sorted(has_bias.items()))
    if key not in _cache:
        _cache[key] = _build(has_bias)
    return _cache[key]


def _chunked(x, width=512):
    """[D, N] -> [P, N//width, S, width] per-chunk contiguous layout."""
    n = x.shape[1]
    nch = n // width
    y = x.reshape(S, P, nch, width).transpose(1, 2, 0, 3)
    return np.ascontiguousarray(y.reshape(P, nch * S * width))


def prepare_inputs(query, key, value, Wq_w, Wq_b, Wk_w, Wk_b, Wv_w, Wv_b,
                   Wo_w, Wo_b):
    bf16 = mybir.dt.np(BF16)
    query = np.asarray(query, dtype=np.float32)
    key = np.asarray(key, dtype=np.float32)
    value = np.asarray(value, dtype=np.float32)
    w = {
        "wq": _chunked(np.asarray(Wq_w, np.float32).T).astype(bf16),
        "wk": _chunked(np.asarray(Wk_w, np.float32).T).astype(bf16),
        "wv": _chunked(np.asarray(Wv_w, np.float32).T).astype(bf16),
        "wo": _chunked(np.asarray(Wo_w, np.float32).T),
    }
    biases = {"bq": np.asarray(Wq_b, np.float32), "bk": np.asarray(Wk_b, np.float32),
              "bv": np.asarray(Wv_b, np.float32), "bo": np.asarray(Wo_b, np.float32)}
    has_bias = {nm: bool(np.any(b)) for nm, b in biases.items()}

    kTh = [[_chunked(np.ascontiguousarray(
                key[b, h * LKH:(h + 1) * LKH, :].T), width=LKH).astype(bf16)
            for h in range(2)] for b in range(B)]
    vTh = [[_chunked(np.ascontiguousarray(
                value[b, h * LKH:(h + 1) * LKH, :].T), width=LKH).astype(bf16)
            for h in range(2)] for b in range(B)]

    in_maps = []
    for c in range(N_CORES):
        b, qc, half = c // (N_CORES // B), c % (N_CORES // B), c % 2
        qslice = query[b, qc * LQ:(qc + 1) * LQ, :]
        m = {
            "qT": _chunked(np.ascontiguousarray(qslice.T)).astype(bf16),
            "kTh": kTh[b][half],
            "vTh": vTh[b][half],
            "goff": (half * P + np.arange(P, dtype=np.int32)).reshape(P, 1),
            **w,
        }
        for nm, hb in has_bias.items():
            if hb:
                m[nm] = biases[nm].reshape(1, D)
        in_maps.append(m)
    return in_maps, has_bias


def gather_output(results):
    out = np.empty((B, L, D), dtype=np.float32)
    for c in range(N_CORES):
        b, qc = c // (N_CORES // B), c % (N_CORES // B)
        oT = results[c]["outT"].reshape(P, S, LQ).transpose(1, 0, 2).reshape(D, LQ)
        out[b, qc * LQ:(qc + 1) * LQ, :] = oT.T
    return out


def kernel(**inputs) -> np.ndarray:
    in_maps, has_bias = prepare_inputs(**inputs)
    nc = _get_program(has_bias)
    res = run_bass_kernel_spmd(nc, in_maps, list(range(N_CORES)))
    return gather_output(res.results)


# revision 26
# speedup vs baseline: 1.0481x; 1.0481x over previous
"""CategoryAttention (softmax over heads axis) on 8 Trainium2 cores.

Sharding: B*L = 2*2048 = 4096 query rows split 8 ways (512 rows/core),
core c handles batch b=c//4, query rows [(c%4)*512, (c%4+1)*512).
The head-axis softmax is local to each (q,k) position, so attention
needs no communication — and K/V projections are not recomputed per
core either: each core projects only HALF of its batch's K/V rows
(even c%2 takes k rows [0,1024), odd takes [1024,2048)) and the halves
are exchanged with 2-rank AllGathers within pairs (0,1),(2,3),(4,5),
(6,7).  Each half is gathered in two 1MB quarters issued as soon as
they are projected, so the collectives overlap the remaining
projections and the first 8 attention k-tiles; gathered quarters land
at rank-invariant global k positions, so the kernel never needs to
know its own rank.  Attention processes k-tiles in arrival order
(0-3, 8-11 from quarter 0; 4-7, 12-15 from quarter 1).

Engine budget per k-tile (the inner loop is overhead-dominated, so few
large ops): PE 6.8us (energy+AV matmuls, AV interleaved between energy
groups so ACT-gated PSUM waits are filled); ACT 8.3us (exp straight
from PSUM in a 4-2-4-2-4 head pattern — alternating 4-bank and 2-bank
psum tiles double-buffer inside the 8-bank budget); DVE ~8us (head-sum
reduce, reciprocal, most of the normalize multiply, AV psum drains);
GPSIMD (2us/op dispatch overhead — give it few big ops) the f32->bf16
recip cast and the rest of the normalize.
"""

import numpy as np
from contextlib import ExitStack

import concourse.bass as bass
import concourse.tile as tile
from concourse import bacc, mybir
from concourse.bass_utils import run_bass_kernel_spmd

F32 = mybir.dt.float32
F32R = mybir.dt.float32r
BF16 = mybir.dt.bfloat16

N_CORES = 8
P = 128
D = 1024          # d_model
S = D // P        # 8 subtiles of the contraction dim
H = 16            # heads
HD = 64           # head dim
B = 2
L = 2048
LQ = L * B // N_CORES   # 512 query rows per core
LK = L                  # key rows per batch
LKH = LK // 2           # 1024 key rows projected per core (own half)
KTS = 128               # k tile
NKT = LK // KTS         # 16
KC = 2                  # k-tiles per AV psum accumulation chunk
SCALE = 1.0 / np.sqrt(HD)

EXP_PAT = (2,) * 8          # heads per ACT exp-from-psum call
ND = 16                     # heads normalized on DVE (rest GPSIMD)
# process k-tiles in gather-arrival order: quarter 0 covers global
# k-tiles {0..3, 8..11}, quarter 1 covers {4..7, 12..15}
KT_ORDER = list(range(16))

GROUPS = [[0, 1], [2, 3], [4, 5], [6, 7]]


def _build(has_bias):
    nc = bacc.Bacc("TRN2", target_bir_lowering=False, debug=False,
                   num_devices=N_CORES)

    def din(name, shape, dt):
        return nc.dram_tensor(name, shape, dt, kind="ExternalInput").ap()

    qT_d = din("qT", (P, S * LQ), BF16)
    kT_d = din("kTh", (P, S * LKH), BF16)
    vT_d = din("vTh", (P, S * LKH), BF16)
    wq_d = din("wq", (P, S * D), BF16)
    wk_d = din("wk", (P, S * D), BF16)
    wv_d = din("wv", (P, S * D), BF16)
    wo_d = din("wo", (P, S * D), F32R)
    goff_d = din("goff", (P, 1), mybir.dt.int32)
    bias_d = {}
    for nm in ("bq", "bk", "bv", "bo"):
        if has_bias[nm]:
            bias_d[nm] = din(nm, (1, D), F32)
    outT_d = nc.dram_tensor("outT", (P, S * LQ), F32, kind="ExternalOutput").ap()

    qT_ap = qT_d.rearrange("p (s q) -> p s q", s=S)
    kT_ap = kT_d.rearrange("p (s k) -> p s k", s=S)
    vT_ap = vT_d.rearrange("p (s k) -> p s k", s=S)
    wq_ap = wq_d.rearrange("p (h s o) -> p h s o", h=2, s=S)
    wk_ap = wk_d.rearrange("p (h s o) -> p h s o", h=2, s=S)
    wv_ap = wv_d.rearrange("p (h s o) -> p h s o", h=2, s=S)
    wo_ap = wo_d.rearrange("p (h s o) -> p h s o", h=2, s=S)
    outT_ap = outT_d.rearrange("p (j q) -> p j q", j=S)

    with tile.TileContext(nc) as tc, ExitStack() as ctx:
        const_pool = ctx.enter_context(tc.tile_pool(name="const", bufs=1))
        qt_pool = ctx.enter_context(tc.tile_pool(name="QT", bufs=1))
        kt_pool = ctx.enter_context(tc.tile_pool(name="KT", bufs=1))
        v_pool = ctx.enter_context(tc.tile_pool(name="V", bufs=1))
        dram_pool = ctx.enter_context(tc.tile_pool(name="dram", bufs=1,
                                                   space="DRAM"))

        any_bias = any(has_bias.values())
        ones_t = None
        if any_bias:
            ones_t = const_pool.tile([1, LKH], F32, tag="ones")
            nc.vector.memset(ones_t[:], 1.0)
        bias_t = {}
        for nm, d_ap in bias_d.items():
            t = const_pool.tile([1, D], F32, tag=f"bias_{nm}")
            nc.sync.dma_start(t[:], d_ap)
            bias_t[nm] = t

        QT_sb = qt_pool.tile([P, S, LQ], BF16)
        KT_sb = kt_pool.tile([P, S, LK], BF16)
        V_sb = v_pool.tile([P, NKT, D], BF16)

        # pair-shared HBM: cores (2k, 2k+1) alias these tensors, so a
        # plain DMA write lands in the peer's address space too.  Each core
        # scatters its K/V half to its own slot (row indices fed from the
        # host via goff — rank-dependent data, rank-oblivious program),
        # then a 2-byte pair AllGather acts as the write barrier before
        # the canonical-layout read-back.
        ksh = dram_pool.tile([2, P, S * LKH], BF16, tag="ksh", name="ksh",
                             addr_space="Shared")
        vsh = dram_pool.tile([2, P, S * LKH], BF16, tag="vsh", name="vsh",
                             addr_space="Shared")
        kbar_i = dram_pool.tile([1, 2], mybir.dt.uint8, tag="kbi", name="kbi")
        kbar_o = dram_pool.tile([2, 2], mybir.dt.uint8, tag="kbo", name="kbo")
        vbar_i = dram_pool.tile([1, 2], mybir.dt.uint8, tag="vbi", name="vbi")
        vbar_o = dram_pool.tile([2, 2], mybir.dt.uint8, tag="vbo", name="vbo")
        goff_t = const_pool.tile([P, 1], mybir.dt.int32, tag="goff")
        nc.sync.dma_start(goff_t[:], goff_d)
        bar_sb = const_pool.tile([1, 2], mybir.dt.uint8, tag="barsb")
        bscr = const_pool.tile([1, 4], mybir.dt.uint8, tag="bscr")
        nc.vector.memset(bar_sb[:], 0)

        def bias_mm(ps_t, bias_name, o0, n_sz, o_on_partitions):
            if o_on_partitions:
                nc.tensor.matmul(ps_t, lhsT=bias_t[bias_name][0:1, o0:o0 + P],
                                 rhs=ones_t[0:1, :n_sz], start=False, stop=True)
            else:
                nc.tensor.matmul(ps_t, lhsT=ones_t[0:1, 0:P],
                                 rhs=bias_t[bias_name][0:1, o0:o0 + n_sz],
                                 start=False, stop=True)

        # ------- phase 1: K/V quarter projections + gathers, Q proj -------
        with tc.tile_pool(name="stream", bufs=4) as spool, \
             tc.tile_pool(name="stage", bufs=1) as stpool, \
             tc.tile_pool(name="ppsum", bufs=3, space="PSUM") as ppsum:

            def pp_tile():
                return ppsum.tile([P, 2, 512], F32, tag="pp", name="pp")

            def stream_w(src_ap):
                t = spool.tile([P, S, 512], BF16, tag="w")
                nc.sync.dma_start(t[:], src_ap)
                return t

            def stream_in(src_ap):
                t = spool.tile([P, S, 512], BF16, tag="in")
                nc.sync.dma_start(t[:], src_ap)
                return t

            # K^T half: out o on partitions, own k cols moving
            wk_h = [stream_w(wk_ap[:, 0])]
            kin0 = stream_in(kT_ap[:, :, 0:512])
            wk_h.append(stream_w(wk_ap[:, 1]))
            wv_h = None
            Kst = stpool.tile([P, S, LKH], BF16, tag="kst")
            Vst = stpool.tile([P, S, LKH], BF16, tag="vst")
            for qh in range(2):
                kin = kin0 if qh == 0 else \
                    stream_in(kT_ap[:, :, qh * 512:(qh + 1) * 512])
                for jp in range(4):
                    ps = pp_tile()
                    for jj in range(2):
                        j = jp * 2 + jj
                        w_t = wk_h[j // 4]
                        jl = j % 4
                        for s in range(S):
                            nc.tensor.matmul(
                                ps[:, jj, :],
                                lhsT=w_t[:, s, jl * P:(jl + 1) * P],
                                rhs=kin[:, s, :],
                                start=(s == 0),
                                stop=(s == S - 1 and not has_bias["bk"]),
                            )
                        if has_bias["bk"]:
                            bias_mm(ps[:, jj, :], "bk", j * P, 512, True)
                    nc.scalar.copy(
                        Kst[:, jp * 2:(jp + 1) * 2, qh * 512:(qh + 1) * 512],
                        ps[:])
            # scatter own K half into the pair-shared buffer, then barrier
            nc.gpsimd.indirect_dma_start(
                out=ksh[:].rearrange("a p n -> (a p) n"),
                out_offset=bass.IndirectOffsetOnAxis(ap=goff_t[:, 0:1], axis=0),
                in_=Kst[:].rearrange("p s k -> p (s k)"),
                in_offset=None,
            )

            # V half: own k rows on partitions, d moving
            for qh in range(2):
                if wv_h is None:
                    wv_h = [stream_w(wv_ap[:, wh]) for wh in range(2)]
                vin = stream_in(vT_ap[:, :, qh * 512:(qh + 1) * 512])
                for kc in range(4):
                    ktc = qh * 4 + kc
                    ps = pp_tile()
                    for t in range(2):
                        for s in range(S):
                            nc.tensor.matmul(
                                ps[:, t, :],
                                lhsT=vin[:, s, kc * P:(kc + 1) * P],
                                rhs=wv_h[t][:, s, :],
                                start=(s == 0),
                                stop=(s == S - 1 and not has_bias["bv"]),
                            )
                        if has_bias["bv"]:
                            bias_mm(ps[:, t, :], "bv", t * 512, 512, False)
                    nc.vector.tensor_copy(Vst[:, ktc, :],
                                          ps[:].rearrange("p a b -> p (a b)"))
            nc.gpsimd.indirect_dma_start(
                out=vsh[:].rearrange("a p n -> (a p) n"),
                out_offset=bass.IndirectOffsetOnAxis(ap=goff_t[:, 0:1], axis=0),
                in_=Vst[:].rearrange("p s k -> p (s k)"),
                in_offset=None,
            )
            nc.gpsimd.dma_start(vbar_i[:], bar_sb[:])
            nc.gpsimd.collective_compute(
                "AllGather", mybir.AluOpType.bypass, replica_groups=GROUPS,
                ins=[vbar_i.opt()], outs=[vbar_o.opt()])
            # reading the collective's output puts a wait on the GP queue,
            # fencing the readbacks below behind the pair barrier
            nc.gpsimd.dma_start(bscr[:],
                                vbar_o[:].rearrange("a b -> (a b)")[None, :])
            for r in range(2):
                nc.gpsimd.dma_start(
                    KT_sb[:, :, r * LKH:(r + 1) * LKH],
                    ksh[r].rearrange("p (s k) -> p s k", s=S))
            for r in range(2):
                nc.gpsimd.dma_start(
                    V_sb[:, r * 8:(r + 1) * 8, :],
                    vsh[r].rearrange("p (c d) -> p c d", c=S))

            # Q^T
            wq_h = [stream_w(wq_ap[:, wh]) for wh in range(2)]
            qin = stream_in(qT_ap)
            for jp in range(4):
                ps = pp_tile()
                for jj in range(2):
                    j = jp * 2 + jj
                    w_t = wq_h[j // 4]
                    jl = j % 4
                    for s in range(S):
                        nc.tensor.matmul(
                            ps[:, jj, :],
                            lhsT=w_t[:, s, jl * P:(jl + 1) * P],
                            rhs=qin[:, s, :],
                            start=(s == 0),
                            stop=(s == S - 1 and not has_bias["bq"]),
                        )
                    if has_bias["bq"]:
                        bias_mm(ps[:, jj, :], "bq", j * P, LQ, True)
                nc.scalar.copy(QT_sb[:, jp * 2:(jp + 1) * 2, :], ps[:])

        # ---------------- phase 2: attention (q = 512) ----------------
        ctx_pool = ctx.enter_context(tc.tile_pool(name="ctx", bufs=1))
        e_psum = ctx.enter_context(tc.tile_pool(name="epsum", bufs=1, space="PSUM"))
        av_psum = ctx.enter_context(tc.tile_pool(name="avpsum", bufs=2, space="PSUM"))
        ctx_sb = ctx_pool.tile([P, S, LQ], F32R)
        # attention SBUF pools are phase-scoped so the O-projection pools
        # below can reuse their space; attn gets a 5th buffer for pipeline
        # elasticity (4 is the strict minimum and causes periodic drains)
        ph2 = ExitStack()
        attn_pool = ph2.enter_context(tc.tile_pool(name="attn", bufs=5))
        tree_pool = ph2.enter_context(tc.tile_pool(name="tree", bufs=2))
        den_pool = ph2.enter_context(tc.tile_pool(name="den", bufs=1))
        r_pool = ph2.enter_context(tc.tile_pool(name="r", bufs=2))
        rb_pool = ph2.enter_context(tc.tile_pool(name="rb", bufs=1))
        rbx_pool = ph2.enter_context(tc.tile_pool(name="rbx", bufs=1))

        pend_norm = []  # [(attn_t, r32)] normalize deferred one k-tile

        def flush_norm():
            """Emit rb/rbx/norm for the previous k-tile.  Deferring these a
            tile keeps the GPSIMD queue from blocking on a just-issued DVE
            reciprocal (the cross-engine ping-pong that paced the loop)."""
            if not pend_norm:
                return
            attn_p, r32_p = pend_norm.pop()
            rb = rb_pool.tile([P, LQ], BF16, tag="rb")
            nc.gpsimd.tensor_copy(rb[:], r32_p[:])
            # replicate 1/den 8x via DMA (stride-0 broadcast APs knock the
            # DVE into a slow generic mode; a physical copy keeps the
            # normalize multiplies on the fast fully-contiguous path)
            rbx = rbx_pool.tile([P, 8, LQ], BF16, tag="rbx")
            nc.gpsimd.dma_start(rbx[:], rb[:, None, :].to_broadcast((P, 8, LQ)))
            for hb in range(2):
                nc.vector.tensor_mul(
                    attn_p[:, hb * 8:(hb + 1) * 8, :],
                    attn_p[:, hb * 8:(hb + 1) * 8, :],
                    rbx[:])

        def softmax_kt(kt, av_jobs):
            """Energy -> exp (ACT, straight from psum) -> den -> attn.

            av_jobs: deferred AV emitters for the previous chunk, spliced
            between energy groups so the in-order PE queue has matmuls to
            run while ACT drains each psum tile.
            """
            flush_norm()
            attn_t = attn_pool.tile([P, H, LQ], BF16, tag="attn")
            tr = tree_pool.tile([P, 8, LQ], BF16, tag="t8")
            h0 = 0
            for gi, gsz in enumerate(EXP_PAT):
                eps = e_psum.tile([P, gsz, LQ], F32, tag=f"e{gsz}{gi % 2}")
                for hh in range(gsz):
                    h = h0 + hh
                    j2, p0 = h // 2, HD * (h % 2)
                    nc.tensor.matmul(
                        eps[:, hh, :],
                        lhsT=KT_sb[p0:p0 + HD, j2, kt * KTS:(kt + 1) * KTS],
                        rhs=QT_sb[p0:p0 + HD, j2, :],
                        start=True,
                        stop=True,
                    )
                nc.scalar.activation(attn_t[:, h0:h0 + gsz, :], eps[:],
                                     mybir.ActivationFunctionType.Exp,
                                     scale=float(SCALE))
                h0 += gsz
                if gi == 3:
                    # heads 0-7 exp'd: start their half-tree early (DVE —
                    # GPSIMD degrades badly under SBUF port contention)
                    with nc.allow_low_precision(reason="bf16 head-sum tree"):
                        nc.vector.tensor_add(tr[:, 0:4, :], attn_t[:, 0:4, :],
                                             attn_t[:, 4:8, :])
                if gi in (2, 5) and av_jobs:
                    av_jobs.pop(0)()
            while av_jobs:
                av_jobs.pop(0)()
            with nc.allow_low_precision(reason="bf16 head-sum tree"):
                nc.vector.tensor_add(tr[:, 4:8, :], attn_t[:, 8:12, :],
                                     attn_t[:, 12:16, :])
                nc.vector.tensor_add(tr[:, 0:4, :], tr[:, 0:4, :],
                                     tr[:, 4:8, :])
                nc.vector.tensor_add(tr[:, 4:6, :], tr[:, 0:2, :],
                                     tr[:, 2:4, :])
            den = den_pool.tile([P, LQ], F32, tag="den")
            nc.vector.tensor_add(den[:], tr[:, 4, :], tr[:, 5, :])
            r32 = r_pool.tile([P, LQ], F32, tag="r")
            nc.vector.reciprocal_approx_fast(r32[:], den[:])
            pend_norm.append((attn_t, r32))
            return attn_t

        def av_group(u, kts, attn_list, first):
            """One avp tile: heads 4u..4u+3, full q, over the KC k-tiles."""
            avp = av_psum.tile([P, 2, LQ], F32, tag="av")
            for ci, kt in enumerate(kts):
                for hh in range(4):
                    h = 4 * u + hh
                    i, p0 = hh // 2, HD * (hh % 2)
                    nc.tensor.matmul(
                        avp[p0:p0 + HD, i, :],
                        lhsT=V_sb[:, kt, h * HD:(h + 1) * HD],
                        rhs=attn_list[ci][:, h, :],
                        start=(ci == 0),
                        stop=(ci == len(kts) - 1),
                    )
            if first:
                nc.vector.tensor_copy(ctx_sb[:, 2 * u:2 * u + 2, :],
                                      avp[:, :, :])
            else:
                nc.vector.tensor_add(ctx_sb[:, 2 * u:2 * u + 2, :],
                                     ctx_sb[:, 2 * u:2 * u + 2, :],
                                     avp[:, :, :])

        prev = None  # (kts, attn_list)
        for ch in range(NKT // KC):
            kts = KT_ORDER[ch * KC:(ch + 1) * KC]
            cur = []
            for ci in range(KC):
                jobs = []
                if prev is not None:
                    pk, pl = prev
                    first = (ch == 1)
                    jobs = [
                        (lambda u=u, pk=pk, pl=pl, first=first:
                         av_group(u, pk, pl, first))
                        for u in (2 * ci, 2 * ci + 1)
                    ]
                cur.append(softmax_kt(kts[ci], jobs))
            prev = (kts, cur)
        flush_norm()
        for u in range(4):
            av_group(u, prev[0], prev[1], False)

        ph2.close()

        # ---------------- phase 3: output projection ----------------
        osb_pool = ctx.enter_context(tc.tile_pool(name="osb", bufs=2))
        wo_pool = ctx.enter_context(tc.tile_pool(name="wo", bufs=1))
        for j4 in range(2):
            woh = wo_pool.tile([P, S, 512], F32R, tag="wo")
            nc.sync.dma_start(woh[:], wo_ap[:, j4])
            for j2 in range(2):
                po = e_psum.tile([P, 2, LQ], F32, tag=f"e2{j2}")
                for jj in range(2):
                    j = j4 * 4 + j2 * 2 + jj
                    jl = j2 * 2 + jj
                    for s in range(S):
                        nc.tensor.matmul(
                            po[:, jj, :],
                            lhsT=woh[:, s, jl * P:(jl + 1) * P],
                            rhs=ctx_sb[:, s, :],
                            start=(s == 0),
                            stop=(s == S - 1 and not has_bias["bo"]),
                        )
                    if has_bias["bo"]:
                        bias_mm(po[:, jj, :], "bo", j * P, LQ, True)
                osb = osb_pool.tile([P, 2, LQ], F32, tag="osb")
                nc.scalar.copy(osb[:], po[:])
                j0 = j4 * 4 + j2 * 2
                nc.sync.dma_start(outT_ap[:, j0:j0 + 2, :], osb[:])

    nc.compile()
    return nc


_cache = {}


def _get_program(has_bias):
    key = tuple(sorted(has_bias.items()))
    if key not in _cache:
        _cache[key] = _build(has_bias)
    return _cache[key]


def _chunked(x, width=512):
    """[D, N] -> [P, N//width, S, width] per-chunk contiguous layout."""
    n = x.shape[1]
    nch = n // width
    y = x.reshape(S, P, nch, width).transpose(1, 2, 0, 3)
    return np.ascontiguousarray(y.reshape(P, nch * S * width))


def prepare_inputs(query, key, value, Wq_w, Wq_b, Wk_w, Wk_b, Wv_w, Wv_b,
                   Wo_w, Wo_b):
    bf16 = mybir.dt.np(BF16)
    query = np.asarray(query, dtype=np.float32)
    key = np.asarray(key, dtype=np.float32)
    value = np.asarray(value, dtype=np.float32)
    w = {
        "wq": _chunked(np.asarray(Wq_w, np.float32).T).astype(bf16),
        "wk": _chunked(np.asarray(Wk_w, np.float32).T).astype(bf16),
        "wv": _chunked(np.asarray(Wv_w, np.float32).T).astype(bf16),
        "wo": _chunked(np.asarray(Wo_w, np.float32).T),
    }
    biases = {"bq": np.asarray(Wq_b, np.float32), "bk": np.asarray(Wk_b, np.float32),
              "bv": np.asarray(Wv_b, np.float32), "bo": np.asarray(Wo_b, np.float32)}
    has_bias = {nm: bool(np.any(b)) for nm, b in biases.items()}

    kTh = [[_chunked(np.ascontiguousarray(
                key[b, h * LKH:(h + 1) * LKH, :].T), width=LKH).astype(bf16)
            for h in range(2)] for b in range(B)]
    vTh = [[_chunked(np.ascontiguousarray(
                value[b, h * LKH:(h + 1) * LKH, :].T), width=LKH).astype(bf16)
            for h in range(2)] for b in range(B)]

    in_maps = []
    for c in range(N_CORES):
        b, qc, half = c // (N_CORES // B), c % (N_CORES // B), c % 2
        qslice = query[b, qc * LQ:(qc + 1) * LQ, :]
        m = {
            "qT": _chunked(np.ascontiguousarray(qslice.T)).astype(bf16),
            "kTh": kTh[b][half],
            "vTh": vTh[b][half],
            "goff": (half * P + np.arange(P, dtype=np.int32)).reshape(P, 1),
            **w,
        }
        for nm, hb in has_bias.items():
            if hb:
                m[nm] = biases[nm].reshape(1, D)
        in_maps.append(m)
    return in_maps, has_bias


def gather_output(results):
    out = np.empty((B, L, D), dtype=np.float32)
    for c in range(N_CORES):
        b, qc = c // (N_CORES // B), c % (N_CORES // B)
        oT = results[c]["outT"].reshape(P, S, LQ).transpose(1, 0, 2).reshape(D, LQ)
        out[b, qc * LQ:(qc + 1) * LQ, :] = oT.T
    return out


def kernel(**inputs) -> np.ndarray:
    in_maps, has_bias = prepare_inputs(**inputs)
    nc = _get_program(has_bias)
    res = run_bass_kernel_spmd(nc, in_maps, list(range(N_CORES)))
    return gather_output(res.results)
